# revision 26
# baseline (speedup 1.0000x reference)
"""Attention2d Trainium2 kernel.

Sharding: 16 heads / 8 cores = 2 heads per core, data-parallel over all 4
batches on every core (head sharding minimizes rel_pos traffic: each core
reads only its 2 heads' [N, N] slices). The output projection contracts over
all heads' channels, so each core emits a partial projection output over its
64 channels; the host sums the 8 partials and adds b_proj.

Device pipeline per (batch, head) pair:
  qkv     = wqkv^T @ x_b           (PE, one [96, N] psum group; q pre-scaled;
                                    single ACT bias eviction)
  v^T     via PE transpose in-place at partitions 64-95 (shifted identity)
  S^T[j,i] = k_jchunk^T q          (PE; scores transposed so the softmax
                                    reduce axis (j) lands on partitions)
  p = exp(S^T) * exp(R^T)          (ACT exp; DVE 2x-mode multiply; exp(R^T)
                                    precomputed on host, resident in SBUF)
  att[d,i] accum += [vt|1]^T @ p   (PE, F=512 matmuls, lhsT is the small
                                    [128, 33] operand so LDWEIGHTS is cheap;
                                    the ones column makes row 32 the softmax
                                    denominator; the two 512-wide i halves
                                    go to PE output quadrants 0 and 64 of a
                                    single psum bank)
  rcp     = 1 / bcast(colsum)      (ACT row copy -> PE ones-broadcast ->
                                    DVE reciprocal, all per-pair batched)
  att_sb  = att * rcp              (DVE multiply doubles as psum eviction)
  out_b  += wp^T-slice @ att_sb    (PE; wp replicated at partitions 0-31 and
                                    64-95 to read both att quadrants;
                                    GpSimd evicts psum so ACT/DVE stay free)
"""

import sys

sys.path.insert(0, "/opt/trn_rl_repo")

import numpy as np
import ml_dtypes

import concourse.bass as bass
import concourse.tile as tile
from concourse import mybir, bacc
from concourse.bass_utils import run_bass_kernel_spmd
from concourse.masks import make_identity

B, C = 4, 512
N = 1024  # 32*32 pixels
HEADS, DH = 16, 32
NCORES = 8
HPC = HEADS // NCORES  # heads per core
F16 = mybir.dt.float16
F32 = mybir.dt.float32
AF = mybir.ActivationFunctionType
OP = mybir.AluOpType

_BUILT = None


def build_nc():
    nc = bacc.Bacc("TRN2", target_bir_lowering=False, debug=False, num_devices=NCORES)
    x16 = nc.declare_dram_parameter("x16", [B, C, N], F16, isOutput=False)
    wqkvT = nc.declare_dram_parameter("wqkvT", [C, HPC, 96], F16, isOutput=False)
    bqkv = nc.declare_dram_parameter("bqkv", [96, HPC], F32, isOutput=False)
    wpT = nc.declare_dram_parameter("wpT", [DH, HPC, C], F16, isOutput=False)
    rt = nc.declare_dram_parameter("rt", [HPC, N, N], F16, isOutput=False)
    outp = nc.declare_dram_parameter("outp", [B, C, N], F16, isOutput=True)

    with tile.TileContext(nc) as tc:
        with (
            tc.tile_pool(name="singles", bufs=1) as singles,
            tc.tile_pool(name="work", bufs=2) as work,
            tc.tile_pool(name="ps", bufs=1, space="PSUM") as pspool,
        ):
            # ---- preamble: constants + resident tensors ----
            # identity for v-transpose, placed at partitions 64-95 where the
            # v rows of the qkv psum eviction live
            idv = singles.tile([96, 32], F16)
            make_identity(nc, idv[64:96, :])
            # all-ones; rows 32 and 96 serve as K=1 lhsT for the colsum
            # partition-broadcast matmuls
            ones_bc = singles.tile([128, 32], F16)
            nc.vector.memset(ones_bc[:], 1.0)

            wq_sb = singles.tile([128, 4, HPC, 96], F16)
            nc.sync.dma_start(
                wq_sb[:], wqkvT.rearrange("(cc p) h m -> p cc h m", p=128)
            )
            bq_sb = singles.tile([96, HPC], F32)
            nc.sync.dma_start(bq_sb[:], bqkv[:])
            # wp at partitions 0-31 (i-half 0) and 64-95 (i-half 1)
            wp_sb = singles.tile([128, HPC, C], F16)
            nc.gpsimd.dma_start(wp_sb[0:32], wpT[:])
            nc.gpsimd.dma_start(wp_sb[64:96], wpT[:])

            # input DMAs ordered/split so the first pair can start early:
            # batch 0 x, then head 0's exp(R^T) in jc chunks, then the rest
            xb_sb = singles.tile([128, B, 4, N], F16)
            expRT = [
                singles.tile([128, 8, N], F16, tag=f"expRT{h}", name=f"expRT{h}")
                for h in range(HPC)
            ]
            x16r = x16.rearrange("b (cc p) n -> b p cc n", p=128)
            for cc in range(4):
                nc.sync.dma_start(xb_sb[:, 0, cc], x16r[0, :, cc])
            rtr = rt.rearrange("h (jc p) i -> h p jc i", p=128)
            for jc in range(8):
                nc.sync.dma_start(expRT[0][:, jc], rtr[0, :, jc])
            for jc in range(8):
                nc.gpsimd.dma_start(expRT[1][:, jc], rtr[1, :, jc])
            for b in range(1, B):
                nc.gpsimd.dma_start(xb_sb[:, b], x16r[b])

            # ---- per-pair stage emitters ----
            def emit_qkv_half(b, h, qkv_hold, nn):
                ps_qkv = pspool.tile([96, 512], F32, tag="qps", bufs=2)
                for cc in range(4):
                    nc.tensor.matmul(
                        ps_qkv[:],
                        lhsT=wq_sb[:, cc, h, :],
                        rhs=xb_sb[:, b, cc, 512 * nn : 512 * nn + 512],
                        start=(cc == 0),
                        stop=(cc == 3),
                    )
                # bias-add eviction on DVE: keeps the ACT queue pure exp so
                # nothing downstream queues behind a pair's worth of exps
                nc.vector.tensor_scalar(
                    out=qkv_hold[:, 512 * nn : 512 * nn + 512],
                    in0=ps_qkv[:],
                    scalar1=bq_sb[:, h : h + 1],
                    scalar2=None,
                    op0=OP.add,
                )

            def emit_mid(b, h, qkv_hold, k_sb, deferred, fin_late, inject):
                # v^T chunks via PE transpose (v stays at partitions 64-95)
                vt_ps = pspool.tile([128, 8, 32], F16, tag="vb")
                for jc in range(8):
                    nc.tensor.transpose(
                        vt_ps[:, jc, :],
                        qkv_hold[64:96, 128 * jc : 128 * jc + 128],
                        idv[64:96, :],
                    )
                vt1 = work.tile([128, 8, 34], F16, tag="vt1")
                nc.vector.tensor_copy(vt1[:, :, 0:32], vt_ps[:])
                nc.vector.memset(vt1[:, :, 32:33], 1.0)

                # scores^T -> exp -> *exp(R^T); att accumulates over jc with
                # i-halves in PE output quadrants 0 and 64 of one psum bank.
                # The P@V matmuls run two jc behind the scores matmuls so the
                # PE never waits on the ACT exp / DVE mult chain; deferred
                # proj chunks from the previous batch fill remaining gaps.
                p2 = work.tile([128, 8, N], F16, tag="p2")
                att_ps = pspool.tile([128, 512], F32, tag="att")

                def attv(jc):
                    for nn in range(2):
                        nc.tensor.matmul(
                            att_ps[64 * nn : 64 * nn + 33, :],
                            lhsT=vt1[:, jc, 0:33],
                            rhs=p2[:, jc, 512 * nn : 512 * nn + 512],
                            start=(jc == 0),
                            stop=(jc == 7),
                        )

                for jc in range(8):
                    ps_s = pspool.tile([128, N], F32, tag="scq", bufs=2)
                    for nn in range(2):
                        nc.tensor.matmul(
                            ps_s[:, 512 * nn : 512 * nn + 512],
                            lhsT=k_sb[:, 128 * jc : 128 * jc + 128],
                            rhs=qkv_hold[0:32, 512 * nn : 512 * nn + 512],
                            start=True,
                            stop=True,
                        )
                    nc.scalar.activation(p2[:, jc, :], ps_s[:], AF.Exp)
                    # GpSimd takes two of the eight rel-pos multiplies to
                    # keep the DVE queue short (it is ~5x slower per call
                    # but otherwise idle; lag-2 consumption gives it slack)
                    eng = nc.gpsimd if jc in (4, 5) else nc.vector
                    eng.tensor_tensor(
                        p2[:, jc, :], p2[:, jc, :], expRT[h][:, jc, :], OP.mult
                    )
                    if jc == 1 and fin_late is not None:
                        fin_late()
                    if jc >= 2:
                        attv(jc - 2)
                    if jc in (3, 4) and inject:
                        inject.pop(0)()
                    if jc >= 5 and deferred:
                        deferred.pop(0)()
                attv(6)
                if deferred:
                    deferred.pop(0)()
                attv(7)
                if deferred:
                    deferred.pop(0)()
                return att_ps

            def emit_fin_early(att_ps):
                # denominators: rows 32/96 of att_ps -> sbuf
                cs = work.tile([128, 512], F16, tag="cs")
                for nn in range(2):
                    r = 64 * nn + 32
                    nc.scalar.activation(
                        cs[r : r + 1, :], att_ps[r : r + 1, :], AF.Identity
                    )
                return cs

            def emit_fin_late(att_ps, cs):
                # ones-broadcast across quadrant partitions -> reciprocal ->
                # fused evict-mult
                bc_ps = pspool.tile([128, 512], F32, tag="qps", bufs=2)
                for nn in range(2):
                    r = 64 * nn + 32
                    nc.tensor.matmul(
                        bc_ps[64 * nn : 64 * nn + 32, :],
                        lhsT=ones_bc[r : r + 1, 0:32],
                        rhs=cs[r : r + 1, :],
                        start=True,
                        stop=True,
                        tile_position=(r, 64 * nn),
                    )
                rcp = work.tile([128, 512], F32, tag="rcp")
                nc.vector.reciprocal_approx_fast(rcp[:], bc_ps[:])
                att_sb = work.tile([128, 512], F16, tag="att_sb", bufs=3)
                nc.vector.tensor_tensor(att_sb[:], att_ps[:], rcp[:], OP.mult)
                return att_sb

            def make_proj(b, att_pair):
                # 4 deferred chunks, each: matmuls + eviction + output DMA
                out_sb = work.tile([128, 4, N], F16, tag="out_sb")
                outr = outp[b].rearrange("(oc p) n -> p oc n", p=128)

                def chunk(oc):
                    def run():
                        for nn in range(2):
                            ps_o = pspool.tile([128, 512], F32, tag="qps", bufs=2)
                            for h in range(HPC):
                                nc.tensor.matmul(
                                    ps_o[:],
                                    lhsT=wp_sb[
                                        64 * nn : 64 * nn + 32,
                                        h,
                                        128 * oc : 128 * oc + 128,
                                    ],
                                    rhs=att_pair[h][64 * nn : 64 * nn + 32, :],
                                    start=(h == 0),
                                    stop=(h == 1),
                                )
                            # GPSIMD can't read PSUM; alternate ACT/DVE
                            dst = out_sb[:, oc, 512 * nn : 512 * nn + 512]
                            if oc == 0:
                                nc.scalar.activation(dst, ps_o[:], AF.Identity)
                            else:
                                nc.vector.tensor_copy(dst, ps_o[:])
                        nc.gpsimd.dma_start(outr[:, oc], out_sb[:, oc, :])

                    return run

                return [chunk(oc) for oc in range(4)]

            # ---- main loop, software-pipelined across pairs:
            #  * pair p+1's qkv matmuls + DVE bias-evict + k DMA are injected
            #    into pair p's score loop (jc 3-4) so its inputs are long
            #    ready when its own loop starts
            #  * pair p-1's colsum/broadcast/reciprocal/evict chain runs at
            #    jc 1, its proj chunks at jc>=5, so the PE queue never heads
            #    on a pair-end dependency ----
            pairs = [(b, h) for b in range(B) for h in range(HPC)]
            qkv_tiles = {}
            att_pair = []
            deferred = []

            def make_qkv_inject(idx):
                b, h = pairs[idx]

                def half0():
                    qkv_hold = work.tile([96, N], F16, tag="qkv_hold")
                    qkv_tiles[idx] = [qkv_hold, None]
                    emit_qkv_half(b, h, qkv_hold, 0)

                def half1():
                    qkv_hold = qkv_tiles[idx][0]
                    emit_qkv_half(b, h, qkv_hold, 1)
                    # k to partitions 0-31 so it can pair with q as lhsT;
                    # the Sync queue carries only head-batch/head inputs so
                    # this issues promptly (bulk preamble rides on GpSimd)
                    k_sb = work.tile([32, N], F16, tag="k_sb")
                    nc.sync.dma_start(k_sb[:], qkv_hold[32:64])
                    qkv_tiles[idx][1] = k_sb

                return [half0, half1]

            def make_fin_late(pb, ph, patt, cs):
                def run():
                    att_sb = emit_fin_late(patt, cs)
                    att_pair.append(att_sb)
                    if ph == HPC - 1:
                        deferred.extend(make_proj(pb, list(att_pair)))
                        att_pair.clear()

                return run

            for fn in make_qkv_inject(0):
                fn()
            pending = None  # (b, h, att_ps)
            fin_late = None
            for idx, (b, h) in enumerate(pairs):
                qkv_hold, k_sb = qkv_tiles.pop(idx)
                if pending is not None:
                    pb, ph, patt = pending
                    cs = emit_fin_early(patt)
                    fin_late = make_fin_late(pb, ph, patt, cs)
                inject = make_qkv_inject(idx + 1) if idx + 1 < len(pairs) else []
                pending = (
                    b,
                    h,
                    emit_mid(b, h, qkv_hold, k_sb, deferred, fin_late, inject),
                )
                fin_late = None
            pb, ph, patt = pending
            cs = emit_fin_early(patt)
            make_fin_late(pb, ph, patt, cs)()
            for fn in deferred:
                fn()

    nc.compile()
    return nc


def _get_nc():
    global _BUILT
    if _BUILT is None:
        _BUILT = build_nc()
    return _BUILT


def _prep_inputs(x, w_qkv, b_qkv, w_proj, b_proj, shared_rel_pos):
    """Host-side sharding/layout prep. Returns per-core input maps."""
    scale = np.float32(DH**-0.5)
    x16 = np.ascontiguousarray(x.reshape(B, C, N)).astype(np.float16)

    wq = w_qkv.reshape(HEADS, 96, C).astype(np.float32).copy()
    wq[:, 0:32, :] *= scale  # fold attention scale into q
    bq = b_qkv.reshape(HEADS, 96).astype(np.float32).copy()
    bq[:, 0:32] *= scale

    in_maps = []
    for g in range(NCORES):
        hh = [HPC * g + h for h in range(HPC)]
        wqkvT = np.ascontiguousarray(
            wq[hh].transpose(2, 0, 1).astype(np.float16)
        )  # [C, HPC, 96]
        bqkv = np.ascontiguousarray(bq[hh].T)  # [96, HPC]
        # w_proj columns for this core's heads, [DH, HPC, C]
        wp = w_proj[:, 64 * g : 64 * (g + 1)].astype(np.float32)  # [C, 64]
        wpT = np.ascontiguousarray(
            wp.T.reshape(HPC, DH, C).transpose(1, 0, 2).astype(np.float16)
        )
        rt = np.ascontiguousarray(
            np.exp(shared_rel_pos[0, hh].transpose(0, 2, 1).astype(np.float32))
        ).astype(np.float16)  # [HPC, N, N] = exp(R^T) per head
        in_maps.append(
            {"x16": x16, "wqkvT": wqkvT, "bqkv": bqkv, "wpT": wpT, "rt": rt}
        )
    return in_maps


def kernel(x, w_qkv, b_qkv, w_proj, b_proj, shared_rel_pos, _trace=False):
    nc = _get_nc()
    in_maps = _prep_inputs(x, w_qkv, b_qkv, w_proj, b_proj, shared_rel_pos)
    res = run_bass_kernel_spmd(nc, in_maps, list(range(NCORES)), trace=_trace)
    kernel.last_result = res
    out = np.zeros((B, C, N), np.float32)
    for g in range(NCORES):
        out += res.results[g]["outp"].astype(np.float32)
    out += b_proj.astype(np.float32)[None, :, None]
    return out.reshape(B, C, 32, 32).astype(np.float32)


# revision 29
# speedup vs baseline: 1.0826x; 1.0826x over previous
"""Attention2d Trainium2 kernel.

Sharding: 16 heads / 8 cores = 2 heads per core, data-parallel over all 4
batches on every core (head sharding minimizes rel_pos traffic: each core
reads only its 2 heads' [N, N] slices). The output projection contracts over
all heads' channels, so each core emits a partial projection output over its
64 channels; the host sums the 8 partials and adds b_proj.

Device pipeline per (batch, head) pair:
  qkv     = wqkv^T @ x_b           (PE, one [96, N] psum group; q pre-scaled;
                                    single ACT bias eviction)
  v^T     via PE transpose in-place at partitions 64-95 (shifted identity)
  S^T[j,i] = k_jchunk^T q          (PE; scores transposed so the softmax
                                    reduce axis (j) lands on partitions)
  p = exp(S^T) * exp(R^T)          (ACT exp; DVE 2x-mode multiply; exp(R^T)
                                    precomputed on host, resident in SBUF)
  att[d,i] accum += [vt|1]^T @ p   (PE, F=512 matmuls, lhsT is the small
                                    [128, 33] operand so LDWEIGHTS is cheap;
                                    the ones column makes row 32 the softmax
                                    denominator; the two 512-wide i halves
                                    go to PE output quadrants 0 and 64 of a
                                    single psum bank)
  rcp     = 1 / bcast(colsum)      (ACT row copy -> PE ones-broadcast ->
                                    DVE reciprocal, all per-pair batched)
  att_sb  = att * rcp              (DVE multiply doubles as psum eviction)
  out_b  += wp^T-slice @ att_sb    (PE; wp replicated at partitions 0-31 and
                                    64-95 to read both att quadrants;
                                    GpSimd evicts psum so ACT/DVE stay free)
"""

import sys

sys.path.insert(0, "/opt/trn_rl_repo")

import numpy as np
import ml_dtypes

import concourse.bass as bass
import concourse.tile as tile
from concourse import mybir, bacc
from concourse.bass_utils import run_bass_kernel_spmd
from concourse.masks import make_identity

B, C = 4, 512
N = 1024  # 32*32 pixels
HEADS, DH = 16, 32
NCORES = 8
HPC = HEADS // NCORES  # heads per core
F16 = mybir.dt.float16
F32 = mybir.dt.float32
AF = mybir.ActivationFunctionType
OP = mybir.AluOpType

_BUILT = None


def build_nc():
    nc = bacc.Bacc("TRN2", target_bir_lowering=False, debug=False, num_devices=NCORES)
    x16 = nc.declare_dram_parameter("x16", [B, C, N], F16, isOutput=False)
    wqkvT = nc.declare_dram_parameter("wqkvT", [C, HPC, 96], F16, isOutput=False)
    bqkv = nc.declare_dram_parameter("bqkv", [96, HPC], F32, isOutput=False)
    wpT = nc.declare_dram_parameter("wpT", [DH, HPC, C], F16, isOutput=False)
    rt = nc.declare_dram_parameter("rt", [HPC, N, N], F16, isOutput=False)
    outp = nc.declare_dram_parameter("outp", [B, C, N], F16, isOutput=True)

    with tile.TileContext(nc) as tc:
        with (
            tc.tile_pool(name="singles", bufs=1) as singles,
            tc.tile_pool(name="work", bufs=2) as work,
            tc.tile_pool(name="ps", bufs=1, space="PSUM") as pspool,
        ):
            # ---- preamble: constants + resident tensors ----
            # identity for v-transpose, placed at partitions 64-95 where the
            # v rows of the qkv psum eviction live
            idv = singles.tile([96, 32], F16)
            make_identity(nc, idv[64:96, :])
            # all-ones; rows 32 and 96 serve as K=1 lhsT for the colsum
            # partition-broadcast matmuls
            ones_bc = singles.tile([128, 32], F16)
            nc.vector.memset(ones_bc[:], 1.0)

            wq_sb = singles.tile([128, 4, HPC, 96], F16)
            nc.sync.dma_start(
                wq_sb[:], wqkvT.rearrange("(cc p) h m -> p cc h m", p=128)
            )
            bq_sb = singles.tile([96, HPC], F32)
            nc.sync.dma_start(bq_sb[:], bqkv[:])
            # wp at partitions 0-31 (i-half 0) and 64-95 (i-half 1)
            wp_sb = singles.tile([128, HPC, C], F16)
            nc.sync.dma_start(wp_sb[0:32], wpT[:])
            nc.sync.dma_start(wp_sb[64:96], wpT[:])

            # input DMAs ordered/split so the first pair can start early:
            # batch 0 x, then head 0's exp(R^T) in jc chunks, then the rest
            xb_sb = singles.tile([128, B, 4, N], F16)
            expRT = [
                singles.tile([128, 8, N], F16, tag=f"expRT{h}", name=f"expRT{h}")
                for h in range(HPC)
            ]
            x16r = x16.rearrange("b (cc p) n -> b p cc n", p=128)
            for cc in range(4):
                nc.sync.dma_start(xb_sb[:, 0, cc], x16r[0, :, cc])
            rtr = rt.rearrange("h (jc p) i -> h p jc i", p=128)
            for h in range(HPC):
                for jc in range(8):
                    nc.sync.dma_start(expRT[h][:, jc], rtr[h, :, jc])
            for b in range(1, B):
                nc.sync.dma_start(xb_sb[:, b], x16r[b])

            # ---- per-pair stage emitters ----
            def emit_qkv_half(b, h, qkv_hold, nn):
                ps_qkv = pspool.tile([96, 512], F32, tag="qps", bufs=2)
                for cc in range(4):
                    nc.tensor.matmul(
                        ps_qkv[:],
                        lhsT=wq_sb[:, cc, h, :],
                        rhs=xb_sb[:, b, cc, 512 * nn : 512 * nn + 512],
                        start=(cc == 0),
                        stop=(cc == 3),
                    )
                # bias-add eviction on DVE: keeps the ACT queue pure exp so
                # nothing downstream queues behind a pair's worth of exps
                nc.vector.tensor_scalar(
                    out=qkv_hold[:, 512 * nn : 512 * nn + 512],
                    in0=ps_qkv[:],
                    scalar1=bq_sb[:, h : h + 1],
                    scalar2=None,
                    op0=OP.add,
                )

            def emit_mid(b, h, qkv_hold, k_sb, deferred, fin_late, inject):
                # v^T chunks via PE transpose (v stays at partitions 64-95)
                vt_ps = pspool.tile([128, 8, 32], F16, tag="vb")
                for jc in range(8):
                    nc.tensor.transpose(
                        vt_ps[:, jc, :],
                        qkv_hold[64:96, 128 * jc : 128 * jc + 128],
                        idv[64:96, :],
                    )
                vt1 = work.tile([128, 8, 34], F16, tag="vt1")
                nc.vector.tensor_copy(vt1[:, :, 0:32], vt_ps[:])
                nc.vector.memset(vt1[:, :, 32:33], 1.0)

                # scores^T -> exp -> *exp(R^T); att accumulates over jc with
                # i-halves in PE output quadrants 0 and 64 of one psum bank.
                # The P@V matmuls run two jc behind the scores matmuls so the
                # PE never waits on the ACT exp / DVE mult chain; deferred
                # proj chunks from the previous batch fill remaining gaps.
                p2 = work.tile([128, 8, N], F16, tag="p2")
                att_ps = pspool.tile([128, 512], F32, tag="att")

                def attv(jc):
                    for nn in range(2):
                        nc.tensor.matmul(
                            att_ps[64 * nn : 64 * nn + 33, :],
                            lhsT=vt1[:, jc, 0:33],
                            rhs=p2[:, jc, 512 * nn : 512 * nn + 512],
                            start=(jc == 0),
                            stop=(jc == 7),
                        )

                for jc in range(8):
                    ps_s = pspool.tile([128, N], F32, tag="scq", bufs=2)
                    for nn in range(2):
                        nc.tensor.matmul(
                            ps_s[:, 512 * nn : 512 * nn + 512],
                            lhsT=k_sb[:, 128 * jc : 128 * jc + 128],
                            rhs=qkv_hold[0:32, 512 * nn : 512 * nn + 512],
                            start=True,
                            stop=True,
                        )
                    nc.scalar.activation(p2[:, jc, :], ps_s[:], AF.Exp)
                    # GpSimd takes two of the eight rel-pos multiplies to
                    # keep the DVE queue short (it is ~5x slower per call
                    # but otherwise idle; lag-2 consumption gives it slack)
                    eng = nc.gpsimd if jc in (4, 5) else nc.vector
                    eng.tensor_tensor(
                        p2[:, jc, :], p2[:, jc, :], expRT[h][:, jc, :], OP.mult
                    )
                    if jc == 1 and fin_late is not None:
                        fin_late()
                    if jc >= 2:
                        attv(jc - 2)
                    if jc in (3, 4) and inject:
                        inject.pop(0)()
                    if jc >= 5 and deferred:
                        deferred.pop(0)()
                attv(6)
                if deferred:
                    deferred.pop(0)()
                attv(7)
                if deferred:
                    deferred.pop(0)()
                return att_ps

            def emit_fin_early(att_ps):
                # denominators: rows 32/96 of att_ps -> sbuf
                cs = work.tile([128, 512], F16, tag="cs")
                for nn in range(2):
                    r = 64 * nn + 32
                    nc.scalar.activation(
                        cs[r : r + 1, :], att_ps[r : r + 1, :], AF.Identity
                    )
                return cs

            def emit_fin_late(att_ps, cs):
                # ones-broadcast across quadrant partitions -> reciprocal ->
                # fused evict-mult
                bc_ps = pspool.tile([128, 512], F32, tag="qps", bufs=2)
                for nn in range(2):
                    r = 64 * nn + 32
                    nc.tensor.matmul(
                        bc_ps[64 * nn : 64 * nn + 32, :],
                        lhsT=ones_bc[r : r + 1, 0:32],
                        rhs=cs[r : r + 1, :],
                        start=True,
                        stop=True,
                        tile_position=(r, 64 * nn),
                    )
                rcp = work.tile([128, 512], F32, tag="rcp")
                nc.vector.reciprocal_approx_fast(rcp[:], bc_ps[:])
                att_sb = work.tile([128, 512], F16, tag="att_sb", bufs=3)
                nc.vector.tensor_tensor(att_sb[:], att_ps[:], rcp[:], OP.mult)
                return att_sb

            def make_proj(b, att_pair):
                # 4 deferred chunks, each: matmuls + eviction + output DMA
                out_sb = work.tile([128, 4, N], F16, tag="out_sb")
                outr = outp[b].rearrange("(oc p) n -> p oc n", p=128)

                def chunk(oc):
                    def run():
                        for nn in range(2):
                            ps_o = pspool.tile([128, 512], F32, tag="qps", bufs=2)
                            for h in range(HPC):
                                nc.tensor.matmul(
                                    ps_o[:],
                                    lhsT=wp_sb[
                                        64 * nn : 64 * nn + 32,
                                        h,
                                        128 * oc : 128 * oc + 128,
                                    ],
                                    rhs=att_pair[h][64 * nn : 64 * nn + 32, :],
                                    start=(h == 0),
                                    stop=(h == 1),
                                )
                            # GPSIMD can't read PSUM; alternate ACT/DVE
                            dst = out_sb[:, oc, 512 * nn : 512 * nn + 512]
                            if oc == 0:
                                nc.scalar.activation(dst, ps_o[:], AF.Identity)
                            else:
                                nc.vector.tensor_copy(dst, ps_o[:])
                        nc.gpsimd.dma_start(outr[:, oc], out_sb[:, oc, :])

                    return run

                return [chunk(oc) for oc in range(4)]

            # ---- main loop, software-pipelined across pairs:
            #  * pair p+1's qkv matmuls + DVE bias-evict + k DMA are injected
            #    into pair p's score loop (jc 3-4) so its inputs are long
            #    ready when its own loop starts
            #  * pair p-1's colsum/broadcast/reciprocal/evict chain runs at
            #    jc 1, its proj chunks at jc>=5, so the PE queue never heads
            #    on a pair-end dependency ----
            pairs = [(b, h) for b in range(B) for h in range(HPC)]
            qkv_tiles = {}
            att_pair = []
            deferred = []

            def make_qkv_inject(idx):
                b, h = pairs[idx]

                def half0():
                    qkv_hold = work.tile([96, N], F16, tag="qkv_hold")
                    qkv_tiles[idx] = [qkv_hold, None]
                    emit_qkv_half(b, h, qkv_hold, 0)

                def half1():
                    qkv_hold = qkv_tiles[idx][0]
                    emit_qkv_half(b, h, qkv_hold, 1)
                    # k to partitions 0-31 so it can pair with q as lhsT;
                    # issued from the GpSimd queue so it never waits behind
                    # bulk input DMAs on the Sync queue
                    k_sb = work.tile([32, N], F16, tag="k_sb")
                    nc.gpsimd.dma_start(k_sb[:], qkv_hold[32:64])
                    qkv_tiles[idx][1] = k_sb

                return [half0, half1]

            def make_fin_late(pb, ph, patt, cs):
                def run():
                    att_sb = emit_fin_late(patt, cs)
                    att_pair.append(att_sb)
                    if ph == HPC - 1:
                        deferred.extend(make_proj(pb, list(att_pair)))
                        att_pair.clear()

                return run

            for fn in make_qkv_inject(0):
                fn()
            pending = None  # (b, h, att_ps)
            fin_late = None
            for idx, (b, h) in enumerate(pairs):
                qkv_hold, k_sb = qkv_tiles.pop(idx)
                if pending is not None:
                    pb, ph, patt = pending
                    cs = emit_fin_early(patt)
                    fin_late = make_fin_late(pb, ph, patt, cs)
                inject = make_qkv_inject(idx + 1) if idx + 1 < len(pairs) else []
                pending = (
                    b,
                    h,
                    emit_mid(b, h, qkv_hold, k_sb, deferred, fin_late, inject),
                )
                fin_late = None
            pb, ph, patt = pending
            cs = emit_fin_early(patt)
            make_fin_late(pb, ph, patt, cs)()
            for fn in deferred:
                fn()

    nc.compile()
    return nc


def _get_nc():
    global _BUILT
    if _BUILT is None:
        _BUILT = build_nc()
    return _BUILT


def _prep_inputs(x, w_qkv, b_qkv, w_proj, b_proj, shared_rel_pos):
    """Host-side sharding/layout prep. Returns per-core input maps."""
    scale = np.float32(DH**-0.5)
    x16 = np.ascontiguousarray(x.reshape(B, C, N)).astype(np.float16)

    wq = w_qkv.reshape(HEADS, 96, C).astype(np.float32).copy()
    wq[:, 0:32, :] *= scale  # fold attention scale into q
    bq = b_qkv.reshape(HEADS, 96).astype(np.float32).copy()
    bq[:, 0:32] *= scale

    in_maps = []
    for g in range(NCORES):
        hh = [HPC * g + h for h in range(HPC)]
        wqkvT = np.ascontiguousarray(
            wq[hh].transpose(2, 0, 1).astype(np.float16)
        )  # [C, HPC, 96]
        bqkv = np.ascontiguousarray(bq[hh].T)  # [96, HPC]
        # w_proj columns for this core's heads, [DH, HPC, C]
        wp = w_proj[:, 64 * g : 64 * (g + 1)].astype(np.float32)  # [C, 64]
        wpT = np.ascontiguousarray(
            wp.T.reshape(HPC, DH, C).transpose(1, 0, 2).astype(np.float16)
        )
        rt = np.ascontiguousarray(
            np.exp(shared_rel_pos[0, hh].transpose(0, 2, 1).astype(np.float32))
        ).astype(np.float16)  # [HPC, N, N] = exp(R^T) per head
        in_maps.append(
            {"x16": x16, "wqkvT": wqkvT, "bqkv": bqkv, "wpT": wpT, "rt": rt}
        )
    return in_maps


def kernel(x, w_qkv, b_qkv, w_proj, b_proj, shared_rel_pos, _trace=False):
    nc = _get_nc()
    in_maps = _prep_inputs(x, w_qkv, b_qkv, w_proj, b_proj, shared_rel_pos)
    res = run_bass_kernel_spmd(nc, in_maps, list(range(NCORES)), trace=_trace)
    kernel.last_result = res
    out = np.zeros((B, C, N), np.float32)
    for g in range(NCORES):
        out += res.results[g]["outp"].astype(np.float32)
    out += b_proj.astype(np.float32)[None, :, None]
    return out.reshape(B, C, 32, 32).astype(np.float32)
